# revision 8
# baseline (speedup 1.0000x reference)
"""Multi-head attention (B=2, T=2048, F=1024, H=16) on 8 trn2 NeuronCores.

Sharding: tensor-parallel over heads — 2 heads per core. Each core computes
Q^T/K^T/V projections for its head pair (column-sliced Wq/Wk/Wv), runs
attention, and a row-sliced output projection producing a partial (B,T,F)
output; the host sums the 8 partials and adds bo.

Layout trick: everything is computed transposed (Q^T, K^T, S^T = K Q^T,
ctx^T) so that no on-chip transposes are needed:
  - S^T tiles [t2, t1] keep softmax's reduction on the PE (a ones-column
    appended to V makes the softmax denominator fall out of the PV matmul).
  - ctx^T chunks are exactly the stationary operand the output projection
    wants.
Matmul operands use float32r (~13-bit mantissa rounding, full-rate PE).
"""

import numpy as np

import concourse.mybir as mybir
import concourse.tile as tile
from concourse import bacc
from concourse.bass_utils import run_bass_kernel_spmd

B, T, F = 2, 2048, 1024
H, DK = 16, 64
NCORES = 8
HPC = H // NCORES          # heads per core
HD = HPC * DK              # 128 head dims per core
KT_ = F // 128             # 8 contraction tiles for projections
T1C = 512                  # t1 chunk (one psum bank)
NT1 = T // T1C             # 4
NT2 = T // 128             # 16 t2 tiles

f32 = mybir.dt.float32
f32r = mybir.dt.float32r
EXP = mybir.ActivationFunctionType.Exp
MULT = mybir.AluOpType.mult


def build_nc(include_bias: bool):
    nc = bacc.Bacc("TRN2", target_bir_lowering=False)

    xqT = nc.dram_tensor("xqT", [B, F, T], f32r, kind="ExternalInput")
    xkT = nc.dram_tensor("xkT", [B, F, T], f32r, kind="ExternalInput")
    xvT = nc.dram_tensor("xvT", [B, F, T], f32r, kind="ExternalInput")
    wq = nc.dram_tensor("wq", [F, HD], f32r, kind="ExternalInput")
    wk = nc.dram_tensor("wk", [F, HD], f32r, kind="ExternalInput")
    wv = nc.dram_tensor("wv", [F, HD], f32r, kind="ExternalInput")
    wo = nc.dram_tensor("wo", [HD, F], f32r, kind="ExternalInput")
    if include_bias:
        bq = nc.dram_tensor("bq", [1, HD], f32r, kind="ExternalInput")
        bk = nc.dram_tensor("bk", [1, HD], f32r, kind="ExternalInput")
        bv = nc.dram_tensor("bv", [1, HD], f32r, kind="ExternalInput")
    out = nc.dram_tensor("out", [B, T, F], f32, kind="ExternalOutput")

    with tile.TileContext(nc) as tc:
        with (
            tc.tile_pool(name="const", bufs=1) as cpool,
            tc.tile_pool(name="xs", bufs=9) as xpool,
            tc.tile_pool(name="work", bufs=1) as wpool,
            tc.tile_pool(name="psum", bufs=1, space="PSUM") as psum,
        ):
            # ---- constants / weights resident in SBUF ----
            wq_s = cpool.tile([128, KT_, HD], f32r, tag="wq")
            wk_s = cpool.tile([128, KT_, HD], f32r, tag="wk")
            wv_s = cpool.tile([128, KT_, HD], f32r, tag="wv")
            wo_s = cpool.tile([HD, F], f32r, tag="wo")
            nc.sync.dma_start(wq_s[:], wq.rearrange("(k p) m -> p k m", p=128))
            nc.sync.dma_start(wk_s[:], wk.rearrange("(k p) m -> p k m", p=128))
            nc.sync.dma_start(wv_s[:], wv.rearrange("(k p) m -> p k m", p=128))
            nc.sync.dma_start(wo_s[:], wo[:])

            with nc.allow_low_precision(reason="fp32r matmul operand rounding"):
                # [1, 64] of ones: stationary for the 1/sum broadcast matmul
                ones64_f = wpool.tile([1, 64], f32, tag="c_f")
                nc.vector.memset(ones64_f[:], 1.0)
                ones64 = cpool.tile([1, 64], f32r, tag="ones64")
                nc.vector.tensor_copy(ones64[:], ones64_f[:])
                # ones column pair for V1 (written into cols 64 and 129)
                onescol_f = wpool.tile([128, 2], f32, tag="c_f2")
                nc.vector.memset(onescol_f[:], 1.0)
                onescol = cpool.tile([128, 2], f32r, tag="onescol")
                nc.vector.tensor_copy(onescol[:], onescol_f[:])
                if include_bias:
                    bq_s = cpool.tile([1, HD], f32r, tag="bq")
                    bk_s = cpool.tile([1, HD], f32r, tag="bk")
                    bv_s = cpool.tile([1, HD], f32r, tag="bv")
                    nc.sync.dma_start(bq_s[:], bq[:])
                    nc.sync.dma_start(bk_s[:], bk[:])
                    nc.sync.dma_start(bv_s[:], bv[:])
                    onesrow_f = wpool.tile([1, T1C], f32, tag="c_f3")
                    nc.vector.memset(onesrow_f[:], 1.0)
                    onesrow = cpool.tile([1, T1C], f32r, tag="onesrow")
                    nc.vector.tensor_copy(onesrow[:], onesrow_f[:])

            for b in range(B):
                # ================= projections =================
                # Q^T and K^T: [HD=128, T] = W.T @ X^T, 8 k-tiles each
                qt = wpool.tile([HD, T], f32r, tag="qt", bufs=2)
                kt = wpool.tile([HD, T], f32r, tag="kt", bufs=2)
                for (xsrc, w_s, dst, bias) in (
                    (xqT, wq_s, qt, "q"),
                    (xkT, wk_s, kt, "k"),
                ):
                    xts = []
                    for k in range(KT_):
                        xt = xpool.tile([128, T], f32r, tag="xt")
                        nc.sync.dma_start(xt[:], xsrc[b, k * 128:(k + 1) * 128, :])
                        xts.append(xt)
                    for n in range(NT1):
                        ps = psum.tile([128, T1C], f32, tag="pa", bufs=2)
                        sl = slice(n * T1C, (n + 1) * T1C)
                        for k in range(KT_):
                            nc.tensor.matmul(
                                ps[:], w_s[:, k, :], xts[k][:, sl],
                                start=(k == 0),
                                stop=(k == KT_ - 1) and not include_bias,
                            )
                        if include_bias:
                            bsrc = bq_s if bias == "q" else bk_s
                            nc.tensor.matmul(ps[:], bsrc[:], onesrow[:],
                                             start=False, stop=True)
                        with nc.allow_low_precision(reason="fp32r rounding"):
                            nc.vector.tensor_copy(dst[:, sl], ps[:])

                # V (token-major) with ones columns:
                # v1[:, tc, 0:65) = [V_h0 | 1], v1[:, tc, 65:130) = [V_h1 | 1]
                v1 = wpool.tile([128, NT2, 2 * 65], f32r, tag="v1", bufs=2)
                xts = []
                for k in range(KT_):
                    xt = xpool.tile([128, T], f32r, tag="xt")
                    nc.sync.dma_start(xt[:], xvT[b, k * 128:(k + 1) * 128, :])
                    xts.append(xt)
                for tcid in range(NT2):
                    ps = psum.tile([128, T1C], f32, tag="pa", bufs=2)
                    sl = slice(tcid * 128, (tcid + 1) * 128)
                    for k in range(KT_):
                        nc.tensor.matmul(
                            ps[:, 0:HD], xts[k][:, sl], wv_s[:, k, :],
                            start=(k == 0),
                            stop=(k == KT_ - 1) and not include_bias,
                        )
                    if include_bias:
                        # out[t, hd] += 1 * bv[hd]
                        nc.tensor.matmul(ps[:, 0:HD], onescol[:, 0:1], bv_s[:],
                                         start=False, stop=True)
                    with nc.allow_low_precision(reason="fp32r rounding"):
                        nc.vector.tensor_copy(v1[:, tcid, 0:64], ps[:, 0:64])
                        nc.vector.tensor_copy(v1[:, tcid, 65:129], ps[:, 64:128])
                        # ones columns at 64 and 129 (stride-65 pair)
                        nc.vector.tensor_copy(
                            v1[:, tcid, 64:130:65], onescol[:]
                        )

                # ================= attention =================
                ctxT = wpool.tile([HD, T], f32r, tag="ctxT", bufs=2)
                for n in range(NT1):
                    sl = slice(n * T1C, (n + 1) * T1C)
                    ctx0 = psum.tile([65, T1C], f32, tag="ctx0")
                    ctx1 = psum.tile([65, T1C], f32, tag="ctx1")
                    for t2 in range(NT2):
                        t2sl = slice(t2 * 128, (t2 + 1) * 128)
                        s0 = psum.tile([128, T1C], f32, tag="st", bufs=4)
                        s1 = psum.tile([128, T1C], f32, tag="st", bufs=4)
                        nc.tensor.matmul(s0[:], kt[0:64, t2sl], qt[0:64, sl],
                                         start=True, stop=True,
                                         tile_position=(0, 0))
                        nc.tensor.matmul(s1[:], kt[64:128, t2sl], qt[64:128, sl],
                                         start=True, stop=True,
                                         tile_position=(64, 0))
                        e0 = wpool.tile([128, T1C], f32r, tag="es", bufs=4)
                        e1 = wpool.tile([128, T1C], f32r, tag="es", bufs=4)
                        with nc.allow_low_precision(reason="fp32r rounding"):
                            nc.scalar.activation(e0[:], s0[:], EXP, scale=0.125)
                            nc.scalar.activation(e1[:], s1[:], EXP, scale=0.125)
                        nc.tensor.matmul(ctx0[:], v1[:, t2, 0:65], e0[:],
                                         start=(t2 == 0), stop=(t2 == NT2 - 1))
                        nc.tensor.matmul(ctx1[:], v1[:, t2, 65:130], e1[:],
                                         start=(t2 == 0), stop=(t2 == NT2 - 1))
                    # normalize: ctxT[:, sl] = ctx[0:64] * (1/sums) broadcast
                    rc0 = wpool.tile([1, T1C], f32, tag="rc0", bufs=2)
                    rc1 = wpool.tile([1, T1C], f32, tag="rc1", bufs=2)
                    nc.vector.tensor_copy(rc0[:], ctx0[64:65, :])
                    nc.vector.tensor_copy(rc1[:], ctx1[64:65, :])
                    rcr0 = wpool.tile([1, T1C], f32r, tag="rcr0", bufs=2)
                    rcr1 = wpool.tile([1, T1C], f32r, tag="rcr1", bufs=2)
                    with nc.allow_low_precision(reason="fp32r rounding"):
                        nc.vector.reciprocal(rcr0[:], rc0[:])
                        nc.vector.reciprocal(rcr1[:], rc1[:])
                    scp0 = psum.tile([64, T1C], f32, tag="st", bufs=4)
                    scp1 = psum.tile([64, T1C], f32, tag="st", bufs=4)
                    nc.tensor.matmul(scp0[:], ones64[:], rcr0[:],
                                     start=True, stop=True)
                    nc.tensor.matmul(scp1[:], ones64[:], rcr1[:],
                                     start=True, stop=True)
                    sc0 = wpool.tile([64, T1C], f32, tag="sc", bufs=4)
                    sc1 = wpool.tile([64, T1C], f32, tag="sc", bufs=4)
                    nc.vector.tensor_copy(sc0[:], scp0[:])
                    nc.vector.tensor_copy(sc1[:], scp1[:])
                    with nc.allow_low_precision(reason="fp32r rounding"):
                        nc.vector.tensor_tensor(ctxT[0:64, sl], ctx0[0:64, :],
                                                sc0[:], MULT)
                        nc.vector.tensor_tensor(ctxT[64:128, sl], ctx1[0:64, :],
                                                sc1[:], MULT)

                # ================= output projection =================
                for tcid in range(NT2):
                    tsl = slice(tcid * 128, (tcid + 1) * 128)
                    ob = wpool.tile([128, F], f32, tag="ob", bufs=3)
                    for half in range(2):
                        po = psum.tile([128, 512], f32, tag="pa", bufs=2)
                        fsl = slice(half * 512, (half + 1) * 512)
                        nc.tensor.matmul(po[:], ctxT[:, tsl], wo_s[:, fsl],
                                         start=True, stop=True)
                        nc.vector.tensor_copy(ob[:, fsl], po[:])
                    nc.sync.dma_start(out[b, tsl, :], ob[:])

    nc.compile()
    return nc


_CACHE = {}


def _get_nc(include_bias: bool):
    if include_bias not in _CACHE:
        _CACHE[include_bias] = build_nc(include_bias)
    return _CACHE[include_bias]


def _reference_fallback(query, key_, value, mask, Wq, bq, Wk, bk, Wv, bv, Wo, bo):
    """Plain numpy fallback (only used if the mask is not all-ones)."""
    q = (query @ Wq + bq).reshape(B, T, H, DK).transpose(0, 2, 1, 3)
    k = (key_ @ Wk + bk).reshape(B, T, H, DK).transpose(0, 2, 1, 3)
    v = (value @ Wv + bv).reshape(B, T, H, DK).transpose(0, 2, 1, 3)
    scores = np.einsum("bhqd,bhkd->bhqk", q, k) / np.sqrt(np.float32(DK))
    scores = np.where(mask[:, None, :, :] > 0, scores,
                      np.float32(-10000.0)).astype(np.float32)
    scores -= scores.max(axis=-1, keepdims=True)
    e = np.exp(scores)
    attn = e / e.sum(axis=-1, keepdims=True)
    x = np.einsum("bhqk,bhkd->bhqd", attn, v)
    x = x.transpose(0, 2, 1, 3).reshape(B, T, F)
    return (x @ Wo + bo).astype(np.float32)


def kernel(**inputs) -> np.ndarray:
    query = np.asarray(inputs["query"], np.float32)
    key_ = np.asarray(inputs.get("key_", inputs.get("key")), np.float32)
    value = np.asarray(inputs["value"], np.float32)
    mask = np.asarray(inputs["mask"])
    Wq, bq = np.asarray(inputs["Wq"], np.float32), np.asarray(inputs["bq"], np.float32)
    Wk, bk = np.asarray(inputs["Wk"], np.float32), np.asarray(inputs["bk"], np.float32)
    Wv, bv = np.asarray(inputs["Wv"], np.float32), np.asarray(inputs["bv"], np.float32)
    Wo, bo = np.asarray(inputs["Wo"], np.float32), np.asarray(inputs["bo"], np.float32)

    if not (mask > 0).all():
        return _reference_fallback(query, key_, value, mask,
                                   Wq, bq, Wk, bk, Wv, bv, Wo, bo)

    include_bias = bool(np.any(bq) or np.any(bk) or np.any(bv))
    nc = _get_nc(include_bias)

    xqT = np.ascontiguousarray(query.transpose(0, 2, 1))
    xkT = np.ascontiguousarray(key_.transpose(0, 2, 1))
    xvT = np.ascontiguousarray(value.transpose(0, 2, 1))

    in_maps = []
    for c in range(NCORES):
        csl = slice(c * HD, (c + 1) * HD)
        m = {
            "xqT": xqT, "xkT": xkT, "xvT": xvT,
            "wq": np.ascontiguousarray(Wq[:, csl]),
            "wk": np.ascontiguousarray(Wk[:, csl]),
            "wv": np.ascontiguousarray(Wv[:, csl]),
            "wo": np.ascontiguousarray(Wo[csl, :]),
        }
        if include_bias:
            m["bq"] = np.ascontiguousarray(bq[None, csl])
            m["bk"] = np.ascontiguousarray(bk[None, csl])
            m["bv"] = np.ascontiguousarray(bv[None, csl])
        in_maps.append(m)

    res = run_bass_kernel_spmd(nc, in_maps, core_ids=list(range(NCORES)))
    total = res.results[0]["out"]
    for c in range(1, NCORES):
        total = total + res.results[c]["out"]
    return (total + bo).astype(np.float32)


# revision 9
# speedup vs baseline: 1.1854x; 1.1854x over previous
"""Multi-head attention (B=2, T=2048, F=1024, H=16) on 8 trn2 NeuronCores.

Sharding: tensor-parallel over heads — 2 heads per core. Each core computes
Q^T/K^T/V^T projections for its head pair (column-sliced Wq/Wk/Wv), runs
attention, and a row-sliced output projection producing a partial (B,T,F)
output; the host sums the 8 partials and adds bo.

Layout: everything is computed transposed (Q^T, K^T, V^T, S^T = K Q^T,
ctx^T) so the only on-chip transposes are 16 cheap 128x128 PE transposes
per batch to build token-major V for the PV matmul. A ones-column appended
to V makes the softmax denominator fall out of the PV matmul for free;
normalization is deferred to after PV (it scales matmul columns linearly).
Matmul operands use float32r (~2^-13 rounding, fp32_mode=HIGH on the PE).
"""

import numpy as np

import concourse.mybir as mybir
import concourse.tile as tile
from concourse import bacc
from concourse.bass_utils import run_bass_kernel_spmd

B, T, F = 2, 2048, 1024
H, DK = 16, 64
NCORES = 8
HPC = H // NCORES          # heads per core
HD = HPC * DK              # 128 head dims per core
KT_ = F // 128             # 8 contraction tiles for projections
TW = 1024                  # t1 window (exp free-dim)
NW = T // TW               # 2 windows
NT2 = T // 128             # 16 t2 tiles

f32 = mybir.dt.float32
f32r = mybir.dt.float32r
EXP = mybir.ActivationFunctionType.Exp
MULT = mybir.AluOpType.mult


def build_nc(include_bias: bool):
    nc = bacc.Bacc("TRN2", target_bir_lowering=False)

    xqT = nc.dram_tensor("xqT", [B, F, T], f32r, kind="ExternalInput")
    xkT = nc.dram_tensor("xkT", [B, F, T], f32r, kind="ExternalInput")
    xvT = nc.dram_tensor("xvT", [B, F, T], f32r, kind="ExternalInput")
    wq = nc.dram_tensor("wq", [F, HD], f32r, kind="ExternalInput")
    wk = nc.dram_tensor("wk", [F, HD], f32r, kind="ExternalInput")
    wv = nc.dram_tensor("wv", [F, HD], f32r, kind="ExternalInput")
    wo = nc.dram_tensor("wo", [HD, F], f32r, kind="ExternalInput")
    ident_in = nc.dram_tensor("ident", [128, 128], f32r, kind="ExternalInput")
    if include_bias:
        bq = nc.dram_tensor("bq", [1, HD], f32r, kind="ExternalInput")
        bk = nc.dram_tensor("bk", [1, HD], f32r, kind="ExternalInput")
        bv = nc.dram_tensor("bv", [1, HD], f32r, kind="ExternalInput")
    out = nc.dram_tensor("out", [B, T, F], f32, kind="ExternalOutput")

    with tile.TileContext(nc) as tc:
        with (
            tc.tile_pool(name="const", bufs=1) as cpool,
            tc.tile_pool(name="xs", bufs=8) as xpool,
            tc.tile_pool(name="work", bufs=1) as wpool,
            tc.tile_pool(name="psum", bufs=1, space="PSUM") as psum,
        ):
            # ---- constants / weights resident in SBUF ----
            wq_s = cpool.tile([128, KT_, HD], f32r, tag="wq")
            wk_s = cpool.tile([128, KT_, HD], f32r, tag="wk")
            wv_s = cpool.tile([128, KT_, HD], f32r, tag="wv")
            wo_s = cpool.tile([HD, F], f32r, tag="wo")
            ident = cpool.tile([128, 128], f32r, tag="ident")
            nc.sync.dma_start(wq_s[:], wq.rearrange("(k p) m -> p k m", p=128))
            nc.sync.dma_start(wk_s[:], wk.rearrange("(k p) m -> p k m", p=128))
            nc.sync.dma_start(wv_s[:], wv.rearrange("(k p) m -> p k m", p=128))
            nc.sync.dma_start(wo_s[:], wo[:])
            nc.sync.dma_start(ident[:], ident_in[:])

            with nc.allow_low_precision(reason="fp32r matmul operand rounding"):
                # [1, 64] of ones: stationary for the 1/sum broadcast matmul
                ones64_f = wpool.tile([1, 64], f32, tag="c_f")
                nc.vector.memset(ones64_f[:], 1.0)
                ones64 = cpool.tile([1, 64], f32r, tag="ones64")
                nc.vector.tensor_copy(ones64[:], ones64_f[:])
                # ones column pair for V1 (written into cols 64 and 129)
                onescol_f = wpool.tile([128, 2], f32, tag="c_f2")
                nc.vector.memset(onescol_f[:], 1.0)
                onescol = cpool.tile([128, 2], f32r, tag="onescol")
                nc.vector.tensor_copy(onescol[:], onescol_f[:])
                if include_bias:
                    bq_s = cpool.tile([1, HD], f32r, tag="bq")
                    bk_s = cpool.tile([1, HD], f32r, tag="bk")
                    bv_s = cpool.tile([1, HD], f32r, tag="bv")
                    nc.sync.dma_start(bq_s[:], bq[:])
                    nc.sync.dma_start(bk_s[:], bk[:])
                    nc.sync.dma_start(bv_s[:], bv[:])
                    onesrow_f = wpool.tile([1, 512], f32, tag="c_f3")
                    nc.vector.memset(onesrow_f[:], 1.0)
                    onesrow = cpool.tile([1, 512], f32r, tag="onesrow")
                    nc.vector.tensor_copy(onesrow[:], onesrow_f[:])

            for b in range(B):
                # ================= projections =================
                # Q^T, K^T, V^T: [HD=128, T] = W.T @ X^T, 8 k-tiles each
                qt = wpool.tile([HD, T], f32r, tag="qt", bufs=2)
                kt = wpool.tile([HD, T], f32r, tag="kt", bufs=2)
                vt = wpool.tile([HD, T], f32r, tag="vt", bufs=2)
                for (xsrc, w_s, dst, bias) in (
                    (xqT, wq_s, qt, "q"),
                    (xkT, wk_s, kt, "k"),
                    (xvT, wv_s, vt, "v"),
                ):
                    xts = []
                    for k in range(KT_):
                        xt = xpool.tile([128, T], f32r, tag="xt")
                        nc.sync.dma_start(xt[:], xsrc[b, k * 128:(k + 1) * 128, :])
                        xts.append(xt)
                    for n in range(T // 512):
                        ps = psum.tile([128, 512], f32, tag="pa", bufs=2)
                        sl = slice(n * 512, (n + 1) * 512)
                        for k in range(KT_):
                            nc.tensor.matmul(
                                ps[:], w_s[:, k, :], xts[k][:, sl],
                                start=(k == 0),
                                stop=(k == KT_ - 1) and not include_bias,
                            )
                        if include_bias:
                            bsrc = {"q": bq_s, "k": bk_s, "v": bv_s}[bias]
                            nc.tensor.matmul(ps[:], bsrc[:], onesrow[:],
                                             start=False, stop=True)
                        with nc.allow_low_precision(reason="fp32r rounding"):
                            nc.vector.tensor_copy(dst[:, sl], ps[:])

                # token-major V with ones columns, via PE transposes of V^T:
                # v1[:, tc, 0:65) = [V_h0 | 1], v1[:, tc, 65:130) = [V_h1 | 1]
                v1 = wpool.tile([128, NT2, 2 * 65], f32r, tag="v1", bufs=2)
                for tcid in range(NT2):
                    pt = psum.tile([128, 128], f32r, tag="pa", bufs=2)
                    tsl = slice(tcid * 128, (tcid + 1) * 128)
                    nc.tensor.transpose(pt[:], vt[:, tsl], ident[:])
                    with nc.allow_low_precision(reason="fp32r rounding"):
                        nc.vector.tensor_copy(v1[:, tcid, 0:64], pt[:, 0:64])
                        nc.vector.tensor_copy(v1[:, tcid, 65:129], pt[:, 64:128])
                        nc.vector.tensor_copy(v1[:, tcid, 64:130:65], onescol[:])

                # ================= attention =================
                ctxT = wpool.tile([HD, T], f32r, tag="ctxT", bufs=2)
                for n in range(NW):
                    wsl = slice(n * TW, (n + 1) * TW)
                    for h in range(HPC):
                        hsl = slice(h * 64, (h + 1) * 64)
                        vsl = slice(h * 65, (h + 1) * 65)
                        tp = (h * 64, 0)
                        ctx = psum.tile([65, TW], f32, tag="ctx", bufs=1)
                        for t2 in range(NT2):
                            t2sl = slice(t2 * 128, (t2 + 1) * 128)
                            s = psum.tile([128, TW], f32, tag="st", bufs=2)
                            es = wpool.tile([128, TW], f32r, tag="es", bufs=3)
                            for q in range(TW // 512):
                                qsl = slice(n * TW + q * 512, n * TW + (q + 1) * 512)
                                nc.tensor.matmul(s[:, q * 512:(q + 1) * 512],
                                                 kt[hsl, t2sl], qt[hsl, qsl],
                                                 start=True, stop=True,
                                                 tile_position=tp)
                            with nc.allow_low_precision(reason="fp32r rounding"):
                                nc.scalar.activation(es[:], s[:], EXP, scale=0.125)
                            for q in range(TW // 512):
                                csl = slice(q * 512, (q + 1) * 512)
                                nc.tensor.matmul(ctx[:, csl], v1[:, t2, vsl],
                                                 es[:, csl],
                                                 start=(t2 == 0),
                                                 stop=(t2 == NT2 - 1))
                        # normalize: ctxT[hsl, wsl] = ctx[0:64] / sums
                        rc = wpool.tile([1, TW], f32, tag="rc", bufs=2)
                        nc.vector.tensor_copy(rc[:], ctx[64:65, :])
                        rcr = wpool.tile([1, TW], f32r, tag="rcr", bufs=2)
                        with nc.allow_low_precision(reason="fp32r rounding"):
                            nc.vector.reciprocal(rcr[:], rc[:])
                        scp = psum.tile([64, TW], f32, tag="st", bufs=2)
                        for q in range(TW // 512):
                            csl = slice(q * 512, (q + 1) * 512)
                            nc.tensor.matmul(scp[:, csl], ones64[:], rcr[:, csl],
                                             start=True, stop=True)
                        sc = wpool.tile([64, TW], f32, tag="sc", bufs=2)
                        nc.vector.tensor_copy(sc[:], scp[:])
                        with nc.allow_low_precision(reason="fp32r rounding"):
                            nc.vector.tensor_tensor(ctxT[hsl, wsl], ctx[0:64, :],
                                                    sc[:], MULT)

                # ================= output projection =================
                for tcid in range(NT2):
                    tsl = slice(tcid * 128, (tcid + 1) * 128)
                    ob = wpool.tile([128, F], f32, tag="ob", bufs=2)
                    for half in range(2):
                        po = psum.tile([128, 512], f32, tag="pa", bufs=2)
                        fsl = slice(half * 512, (half + 1) * 512)
                        nc.tensor.matmul(po[:], ctxT[:, tsl], wo_s[:, fsl],
                                         start=True, stop=True)
                        nc.vector.tensor_copy(ob[:, fsl], po[:])
                    nc.sync.dma_start(out[b, tsl, :], ob[:])

    nc.compile()
    return nc


_CACHE = {}


def _get_nc(include_bias: bool):
    if include_bias not in _CACHE:
        _CACHE[include_bias] = build_nc(include_bias)
    return _CACHE[include_bias]


def _reference_fallback(query, key_, value, mask, Wq, bq, Wk, bk, Wv, bv, Wo, bo):
    """Plain numpy fallback (only used if the mask is not all-ones)."""
    q = (query @ Wq + bq).reshape(B, T, H, DK).transpose(0, 2, 1, 3)
    k = (key_ @ Wk + bk).reshape(B, T, H, DK).transpose(0, 2, 1, 3)
    v = (value @ Wv + bv).reshape(B, T, H, DK).transpose(0, 2, 1, 3)
    scores = np.einsum("bhqd,bhkd->bhqk", q, k) / np.sqrt(np.float32(DK))
    scores = np.where(mask[:, None, :, :] > 0, scores,
                      np.float32(-10000.0)).astype(np.float32)
    scores -= scores.max(axis=-1, keepdims=True)
    e = np.exp(scores)
    attn = e / e.sum(axis=-1, keepdims=True)
    x = np.einsum("bhqk,bhkd->bhqd", attn, v)
    x = x.transpose(0, 2, 1, 3).reshape(B, T, F)
    return (x @ Wo + bo).astype(np.float32)


def make_in_maps(query, key_, value, Wq, Wk, Wv, Wo, bq=None, bk=None, bv=None):
    xqT = np.ascontiguousarray(query.transpose(0, 2, 1))
    xkT = np.ascontiguousarray(key_.transpose(0, 2, 1))
    xvT = np.ascontiguousarray(value.transpose(0, 2, 1))
    ident = np.eye(128, dtype=np.float32)
    in_maps = []
    for c in range(NCORES):
        csl = slice(c * HD, (c + 1) * HD)
        m = {
            "xqT": xqT, "xkT": xkT, "xvT": xvT, "ident": ident,
            "wq": np.ascontiguousarray(Wq[:, csl]),
            "wk": np.ascontiguousarray(Wk[:, csl]),
            "wv": np.ascontiguousarray(Wv[:, csl]),
            "wo": np.ascontiguousarray(Wo[csl, :]),
        }
        if bq is not None:
            m["bq"] = np.ascontiguousarray(bq[None, csl])
            m["bk"] = np.ascontiguousarray(bk[None, csl])
            m["bv"] = np.ascontiguousarray(bv[None, csl])
        in_maps.append(m)
    return in_maps


def kernel(**inputs) -> np.ndarray:
    query = np.asarray(inputs["query"], np.float32)
    key_ = np.asarray(inputs.get("key_", inputs.get("key")), np.float32)
    value = np.asarray(inputs["value"], np.float32)
    mask = np.asarray(inputs["mask"])
    Wq, bq = np.asarray(inputs["Wq"], np.float32), np.asarray(inputs["bq"], np.float32)
    Wk, bk = np.asarray(inputs["Wk"], np.float32), np.asarray(inputs["bk"], np.float32)
    Wv, bv = np.asarray(inputs["Wv"], np.float32), np.asarray(inputs["bv"], np.float32)
    Wo, bo = np.asarray(inputs["Wo"], np.float32), np.asarray(inputs["bo"], np.float32)

    if not (mask > 0).all():
        return _reference_fallback(query, key_, value, mask,
                                   Wq, bq, Wk, bk, Wv, bv, Wo, bo)

    include_bias = bool(np.any(bq) or np.any(bk) or np.any(bv))
    nc = _get_nc(include_bias)
    if include_bias:
        in_maps = make_in_maps(query, key_, value, Wq, Wk, Wv, Wo, bq, bk, bv)
    else:
        in_maps = make_in_maps(query, key_, value, Wq, Wk, Wv, Wo)

    res = run_bass_kernel_spmd(nc, in_maps, core_ids=list(range(NCORES)))
    total = res.results[0]["out"]
    for c in range(1, NCORES):
        total = total + res.results[c]["out"]
    return (total + bo).astype(np.float32)


# revision 11
# speedup vs baseline: 1.3148x; 1.1092x over previous
"""Multi-head attention (B=2, T=2048, F=1024, H=16) on 8 trn2 NeuronCores.

Sharding: tensor-parallel over heads — 2 heads per core. Each core computes
Q^T/K^T/V^T projections for its head pair (column-sliced Wq/Wk/Wv), runs
attention, and a row-sliced output projection producing a partial (B,T,F)
output; the host sums the 8 partials and adds bo.

Layout: everything is computed transposed (Q^T, K^T, V^T, S^T = K Q^T,
ctx^T) so the only on-chip transposes are 16 cheap 128x128 PE transposes
per batch to build token-major V for the PV matmul. A ones-column appended
to V makes the softmax denominator fall out of the PV matmul for free;
normalization is deferred to after PV (it scales matmul columns linearly).

MODE selects the matmul operand dtype for the bulk pipeline:
  "f32r": float32r everywhere (~2^-13 operand rounding) — most accurate.
  "bf16": bfloat16 X/W/Q/K/V/expS (faster PE + half the input DMA);
          softmax-normalization and output projection stay float32r.
Filler matmuls are injected during attention to keep the PE HAM activity
monitor busy (otherwise the 50%-duty attention phase keeps the PE clock
throttled at 1.2 GHz).
"""

import os

import numpy as np

import concourse.mybir as mybir
import concourse.tile as tile
from concourse import bacc
from concourse.bass_utils import run_bass_kernel_spmd

B, T, F = 2, 2048, 1024
H, DK = 16, 64
NCORES = 8
HPC = H // NCORES          # heads per core
HD = HPC * DK              # 128 head dims per core
KT_ = F // 128             # 8 contraction tiles for projections
TW = 1024                  # t1 window (exp free-dim)
NW = T // TW               # 2 windows
NT2 = T // 128             # 16 t2 tiles

f32 = mybir.dt.float32
f32r = mybir.dt.float32r
bf16 = mybir.dt.bfloat16
EXP = mybir.ActivationFunctionType.Exp
MULT = mybir.AluOpType.mult

MODE = os.environ.get("MHA_MODE", "f32r")
FILL = int(os.environ.get("MHA_FILL", "2"))


def build_nc(include_bias: bool, mode: str = MODE, fill: int = FILL):
    mdt = bf16 if mode == "bf16" else f32r
    nc = bacc.Bacc("TRN2", target_bir_lowering=False)

    xqT = nc.dram_tensor("xqT", [B, F, T], mdt, kind="ExternalInput")
    xkT = nc.dram_tensor("xkT", [B, F, T], mdt, kind="ExternalInput")
    xvT = nc.dram_tensor("xvT", [B, F, T], mdt, kind="ExternalInput")
    wq = nc.dram_tensor("wq", [F, HD], mdt, kind="ExternalInput")
    wk = nc.dram_tensor("wk", [F, HD], mdt, kind="ExternalInput")
    wv = nc.dram_tensor("wv", [F, HD], mdt, kind="ExternalInput")
    wo = nc.dram_tensor("wo", [HD, F], f32r, kind="ExternalInput")
    ident_in = nc.dram_tensor("ident", [128, 128], mdt, kind="ExternalInput")
    if include_bias:
        bq = nc.dram_tensor("bq", [1, HD], mdt, kind="ExternalInput")
        bk = nc.dram_tensor("bk", [1, HD], mdt, kind="ExternalInput")
        bv = nc.dram_tensor("bv", [1, HD], mdt, kind="ExternalInput")
    out = nc.dram_tensor("out", [B, T, F], f32, kind="ExternalOutput")

    with tile.TileContext(nc) as tc:
        with (
            tc.tile_pool(name="const", bufs=1) as cpool,
            tc.tile_pool(name="xs", bufs=8) as xpool,
            tc.tile_pool(name="work", bufs=1) as wpool,
            tc.tile_pool(name="psum", bufs=1, space="PSUM") as psum,
        ):
            # ---- constants / weights resident in SBUF ----
            wq_s = cpool.tile([128, KT_, HD], mdt, tag="wq")
            wk_s = cpool.tile([128, KT_, HD], mdt, tag="wk")
            wv_s = cpool.tile([128, KT_, HD], mdt, tag="wv")
            wo_s = cpool.tile([HD, F], f32r, tag="wo")
            ident = cpool.tile([128, 128], mdt, tag="ident")
            nc.sync.dma_start(wq_s[:], wq.rearrange("(k p) m -> p k m", p=128))
            nc.sync.dma_start(wk_s[:], wk.rearrange("(k p) m -> p k m", p=128))
            nc.sync.dma_start(wv_s[:], wv.rearrange("(k p) m -> p k m", p=128))
            nc.sync.dma_start(wo_s[:], wo[:])
            nc.sync.dma_start(ident[:], ident_in[:])

            with nc.allow_low_precision(reason="matmul operand rounding"):
                # [1, 64] of ones: stationary for the 1/sum broadcast matmul
                ones64_f = wpool.tile([1, 64], f32, tag="c_f")
                nc.vector.memset(ones64_f[:], 1.0)
                ones64 = cpool.tile([1, 64], f32r, tag="ones64")
                nc.vector.tensor_copy(ones64[:], ones64_f[:])
                # ones column pair for V1 (written into cols 64 and 129)
                onescol_f = wpool.tile([128, 2], f32, tag="c_f2")
                nc.vector.memset(onescol_f[:], 1.0)
                onescol = cpool.tile([128, 2], mdt, tag="onescol")
                nc.vector.tensor_copy(onescol[:], onescol_f[:])
                if include_bias:
                    bq_s = cpool.tile([1, HD], mdt, tag="bq")
                    bk_s = cpool.tile([1, HD], mdt, tag="bk")
                    bv_s = cpool.tile([1, HD], mdt, tag="bv")
                    nc.sync.dma_start(bq_s[:], bq[:])
                    nc.sync.dma_start(bk_s[:], bk[:])
                    nc.sync.dma_start(bv_s[:], bv[:])
                    onesrow_f = wpool.tile([1, 512], f32, tag="c_f3")
                    nc.vector.memset(onesrow_f[:], 1.0)
                    onesrow = cpool.tile([1, 512], mdt, tag="onesrow")
                    nc.vector.tensor_copy(onesrow[:], onesrow_f[:])

            def filler(n_mm):
                """Dummy matmuls to keep the PE HAM activity window busy."""
                for _ in range(n_mm):
                    pf = psum.tile([128, 512], f32, tag="pa", bufs=2,
                                   name="fill")
                    nc.tensor.matmul(pf[:], wq_s[:, 0, :],
                                     wq_s[:, 0:4, :], start=True, stop=True)

            for b in range(B):
                # ================= projections =================
                # Q^T, K^T, V^T: [HD=128, T] = W.T @ X^T, 8 k-tiles each
                qt = wpool.tile([HD, T], mdt, tag="qt", bufs=2)
                kt = wpool.tile([HD, T], mdt, tag="kt", bufs=2)
                vt = wpool.tile([HD, T], mdt, tag="vt", bufs=2)
                for (xsrc, w_s, dst, bias) in (
                    (xqT, wq_s, qt, "q"),
                    (xkT, wk_s, kt, "k"),
                    (xvT, wv_s, vt, "v"),
                ):
                    xts = []
                    for k in range(KT_):
                        xt = xpool.tile([128, T], mdt, tag="xt")
                        nc.sync.dma_start(xt[:], xsrc[b, k * 128:(k + 1) * 128, :])
                        xts.append(xt)
                    for n in range(T // 512):
                        ps = psum.tile([128, 512], f32, tag="pa", bufs=2)
                        sl = slice(n * 512, (n + 1) * 512)
                        for k in range(KT_):
                            nc.tensor.matmul(
                                ps[:], w_s[:, k, :], xts[k][:, sl],
                                start=(k == 0),
                                stop=(k == KT_ - 1) and not include_bias,
                            )
                        if include_bias:
                            bsrc = {"q": bq_s, "k": bk_s, "v": bv_s}[bias]
                            nc.tensor.matmul(ps[:], bsrc[:], onesrow[:],
                                             start=False, stop=True)
                        with nc.allow_low_precision(reason="rounding"):
                            nc.vector.tensor_copy(dst[:, sl], ps[:])

                # token-major V with ones columns, via PE transposes of V^T:
                # v1[:, tc, 0:65) = [V_h0 | 1], v1[:, tc, 65:130) = [V_h1 | 1]
                v1 = wpool.tile([128, NT2, 2 * 65], mdt, tag="v1", bufs=2)
                for tcid in range(NT2):
                    pt = psum.tile([128, 128], mdt, tag="pa", bufs=2)
                    tsl = slice(tcid * 128, (tcid + 1) * 128)
                    nc.tensor.transpose(pt[:], vt[:, tsl], ident[:])
                    with nc.allow_low_precision(reason="rounding"):
                        nc.vector.tensor_copy(v1[:, tcid, 0:64], pt[:, 0:64])
                        nc.vector.tensor_copy(v1[:, tcid, 65:129], pt[:, 64:128])
                        nc.vector.tensor_copy(v1[:, tcid, 64:130:65], onescol[:])

                # ================= attention =================
                ctxT = wpool.tile([HD, T], f32r, tag="ctxT", bufs=2)
                for n in range(NW):
                    wsl = slice(n * TW, (n + 1) * TW)
                    for h in range(HPC):
                        hsl = slice(h * 64, (h + 1) * 64)
                        vsl = slice(h * 65, (h + 1) * 65)
                        tp = (h * 64, 0)
                        ctx = psum.tile([65, TW], f32, tag="ctx", bufs=1)
                        for t2 in range(NT2):
                            t2sl = slice(t2 * 128, (t2 + 1) * 128)
                            s = psum.tile([128, TW], f32, tag="st", bufs=2)
                            es = wpool.tile([128, TW], mdt, tag="es", bufs=3)
                            for q in range(TW // 512):
                                qsl = slice(n * TW + q * 512, n * TW + (q + 1) * 512)
                                nc.tensor.matmul(s[:, q * 512:(q + 1) * 512],
                                                 kt[hsl, t2sl], qt[hsl, qsl],
                                                 start=True, stop=True,
                                                 tile_position=tp)
                            with nc.allow_low_precision(reason="rounding"):
                                nc.scalar.activation(es[:], s[:], EXP, scale=0.125)
                            for q in range(TW // 512):
                                csl = slice(q * 512, (q + 1) * 512)
                                nc.tensor.matmul(ctx[:, csl], v1[:, t2, vsl],
                                                 es[:, csl],
                                                 start=(t2 == 0),
                                                 stop=(t2 == NT2 - 1))
                            if fill:
                                filler(fill)
                        # normalize: ctxT[hsl, wsl] = ctx[0:64] / sums
                        rc = wpool.tile([1, TW], f32, tag="rc", bufs=2)
                        nc.vector.tensor_copy(rc[:], ctx[64:65, :])
                        rcr = wpool.tile([1, TW], f32r, tag="rcr", bufs=2)
                        with nc.allow_low_precision(reason="rounding"):
                            nc.vector.reciprocal(rcr[:], rc[:])
                        scp = psum.tile([64, TW], f32, tag="st", bufs=2)
                        for q in range(TW // 512):
                            csl = slice(q * 512, (q + 1) * 512)
                            nc.tensor.matmul(scp[:, csl], ones64[:], rcr[:, csl],
                                             start=True, stop=True)
                        sc = wpool.tile([64, TW], f32, tag="sc", bufs=2)
                        nc.vector.tensor_copy(sc[:], scp[:])
                        with nc.allow_low_precision(reason="rounding"):
                            nc.vector.tensor_tensor(ctxT[hsl, wsl], ctx[0:64, :],
                                                    sc[:], MULT)

                # ================= output projection =================
                for tcid in range(NT2):
                    tsl = slice(tcid * 128, (tcid + 1) * 128)
                    ob = wpool.tile([128, F], f32, tag="ob", bufs=2)
                    for half in range(2):
                        po = psum.tile([128, 512], f32, tag="pa", bufs=2)
                        fsl = slice(half * 512, (half + 1) * 512)
                        nc.tensor.matmul(po[:], ctxT[:, tsl], wo_s[:, fsl],
                                         start=True, stop=True)
                        nc.vector.tensor_copy(ob[:, fsl], po[:])
                    nc.sync.dma_start(out[b, tsl, :], ob[:])

    nc.compile()
    return nc


_CACHE = {}


def _get_nc(include_bias: bool):
    key = (include_bias, MODE, FILL)
    if key not in _CACHE:
        _CACHE[key] = build_nc(include_bias)
    return _CACHE[key]


def _reference_fallback(query, key_, value, mask, Wq, bq, Wk, bk, Wv, bv, Wo, bo):
    """Plain numpy fallback (only used if the mask is not all-ones)."""
    q = (query @ Wq + bq).reshape(B, T, H, DK).transpose(0, 2, 1, 3)
    k = (key_ @ Wk + bk).reshape(B, T, H, DK).transpose(0, 2, 1, 3)
    v = (value @ Wv + bv).reshape(B, T, H, DK).transpose(0, 2, 1, 3)
    scores = np.einsum("bhqd,bhkd->bhqk", q, k) / np.sqrt(np.float32(DK))
    scores = np.where(mask[:, None, :, :] > 0, scores,
                      np.float32(-10000.0)).astype(np.float32)
    scores -= scores.max(axis=-1, keepdims=True)
    e = np.exp(scores)
    attn = e / e.sum(axis=-1, keepdims=True)
    x = np.einsum("bhqk,bhkd->bhqd", attn, v)
    x = x.transpose(0, 2, 1, 3).reshape(B, T, F)
    return (x @ Wo + bo).astype(np.float32)


def _mdt_np(arr):
    if MODE == "bf16":
        import ml_dtypes
        return np.ascontiguousarray(arr).astype(ml_dtypes.bfloat16)
    return np.ascontiguousarray(arr)


def make_in_maps(query, key_, value, Wq, Wk, Wv, Wo, bq=None, bk=None, bv=None):
    xqT = _mdt_np(query.transpose(0, 2, 1))
    xkT = _mdt_np(key_.transpose(0, 2, 1))
    xvT = _mdt_np(value.transpose(0, 2, 1))
    ident = _mdt_np(np.eye(128, dtype=np.float32))
    in_maps = []
    for c in range(NCORES):
        csl = slice(c * HD, (c + 1) * HD)
        m = {
            "xqT": xqT, "xkT": xkT, "xvT": xvT, "ident": ident,
            "wq": _mdt_np(Wq[:, csl]),
            "wk": _mdt_np(Wk[:, csl]),
            "wv": _mdt_np(Wv[:, csl]),
            "wo": np.ascontiguousarray(Wo[csl, :]),
        }
        if bq is not None:
            m["bq"] = _mdt_np(bq[None, csl])
            m["bk"] = _mdt_np(bk[None, csl])
            m["bv"] = _mdt_np(bv[None, csl])
        in_maps.append(m)
    return in_maps


def kernel(**inputs) -> np.ndarray:
    query = np.asarray(inputs["query"], np.float32)
    key_ = np.asarray(inputs.get("key_", inputs.get("key")), np.float32)
    value = np.asarray(inputs["value"], np.float32)
    mask = np.asarray(inputs["mask"])
    Wq, bq = np.asarray(inputs["Wq"], np.float32), np.asarray(inputs["bq"], np.float32)
    Wk, bk = np.asarray(inputs["Wk"], np.float32), np.asarray(inputs["bk"], np.float32)
    Wv, bv = np.asarray(inputs["Wv"], np.float32), np.asarray(inputs["bv"], np.float32)
    Wo, bo = np.asarray(inputs["Wo"], np.float32), np.asarray(inputs["bo"], np.float32)

    if not (mask > 0).all():
        return _reference_fallback(query, key_, value, mask,
                                   Wq, bq, Wk, bk, Wv, bv, Wo, bo)

    include_bias = bool(np.any(bq) or np.any(bk) or np.any(bv))
    nc = _get_nc(include_bias)
    if include_bias:
        in_maps = make_in_maps(query, key_, value, Wq, Wk, Wv, Wo, bq, bk, bv)
    else:
        in_maps = make_in_maps(query, key_, value, Wq, Wk, Wv, Wo)

    res = run_bass_kernel_spmd(nc, in_maps, core_ids=list(range(NCORES)))
    total = res.results[0]["out"]
    for c in range(1, NCORES):
        total = total + res.results[c]["out"]
    return (total + bo).astype(np.float32)
